# revision 1
# baseline (speedup 1.0000x reference)
"""Trainium2 Bass kernel for conv-QK causal attention + MLP.

Reference computation (B=4, T=2048, D=512, H=8, DK=DV=64, FS=3):
  q = causal_conv1d(x, Wq) + bq ; k = causal_conv1d(x, Wk) + bk
  v = x @ Wv + bv
  per-head causal attention (softmax(q k^T / 8))
  out = relu(attn @ W1 + b1) @ W2 + b2        -> [B, T, 64]

Sharding: head-parallel, one head per NeuronCore (H == 8 == n_cores).
Each core computes q/k/v and attention for its head over all batches,
its partial attn @ W1[head], then ReduceScatters (x2, pipelined with
compute of the later batches) to sum head partials and shard tokens
8-way for the final relu/W2 epilogue.

On-chip layout is "transposed" (channels on partitions, tokens on the
free axis) so softmax needs no transposes at all:
  St[k, q] = K Q^T computed per 128-row k-block strip; exp on ScalarE
  (no max subtraction: logits are O(1) by construction); causal masking
  via a 0/1 upper-triangular multiply on the diagonal block; P V done as
  O~^T = [V | 1]^T P^T which also accumulates the softmax denominators
  as row 64 of the PSUM accumulator.

dtypes: the whole heavy path (x, projections, QK, P, V, W1/W2) runs in
bf16 (1 cyc/row on the PE; mixing fp32r and bf16 matmuls costs a ~200ns
mode-switch per transition, so the stream is kept uniform); PSUM
accumulation and the softmax denominators stay fp32.  The K=64
attention QK matmuls are row-packed in pairs on array halves (0:64 /
64:128) via partition-replicated Q/K copies, so they run concurrently,
and their issue order is interleaved across the pair.  W1 partials and
the post-reduce-scatter epilogues are emitted a few pair-iterations
after the data chain that feeds them: the PE queue is strictly
in-order, so anything that waits on ACT/DVE/DMA/collective latency
must sit late in the queue or it stalls every matmul behind it.
"""

import ml_dtypes
import numpy as np

import concourse.bass as bass
import concourse.mybir as mybir
import concourse.tile as tile
from concourse import bacc, bass_utils
from concourse.masks import make_identity, make_upper_triangular

B, T, D = 4, 2048, 512
H, DK, DV, FS = 8, 64, 64, 3
NCORES = 8
TP = T + FS - 1          # left-zero-padded time axis (2050)
NDT = D // 128           # d-tiles (4)
NTT = T // 128           # t-tiles (16)
TOK = B * T // NCORES    # 1024 output tokens per core
HTOK = TOK // 2          # tokens per reduce-scatter half

F32 = mybir.dt.float32
F32R = mybir.dt.float32r
BF16 = mybir.dt.bfloat16

_STATE = {}


def _build():
    nc = bacc.Bacc("TRN2", target_bir_lowering=False, debug=False,
                   num_devices=NCORES)

    xtp = nc.dram_tensor("xtp", [B, D, TP], BF16, kind="ExternalInput")
    wqk = nc.dram_tensor("wqk", [FS, NDT, 128, 128], BF16, kind="ExternalInput")
    wv = nc.dram_tensor("wv", [NDT, 128, DV], BF16, kind="ExternalInput")
    w1 = nc.dram_tensor("w1", [DV, 64], BF16, kind="ExternalInput")
    w2 = nc.dram_tensor("w2", [64, 64], BF16, kind="ExternalInput")
    bqk = nc.dram_tensor("bqk", [128, 1], F32, kind="ExternalInput")
    bv = nc.dram_tensor("bv", [64, 1], F32, kind="ExternalInput")
    b1 = nc.dram_tensor("b1", [64, 1], F32, kind="ExternalInput")
    b2 = nc.dram_tensor("b2", [64, 1], F32, kind="ExternalInput")
    out = nc.dram_tensor("out", [TOK, DV], F32, kind="ExternalOutput")

    EXP = mybir.ActivationFunctionType.Exp

    with tile.TileContext(nc) as tc:
        with (
            tc.tile_pool(name="cpool", bufs=1) as cpool,
            tc.tile_pool(name="xpool", bufs=2) as xpool,
            tc.tile_pool(name="qkpool", bufs=2) as qkpool,
            tc.tile_pool(name="vtpool", bufs=2) as vtpool,
            tc.tile_pool(name="vpool", bufs=2) as vpool,
            tc.tile_pool(name="ptpool", bufs=6) as ptpool,
            tc.tile_pool(name="atpool", bufs=2) as atpool,
            tc.tile_pool(name="spool", bufs=2) as spool,
            tc.tile_pool(name="stpool", bufs=2, space="PSUM") as stpool,
            tc.tile_pool(name="opool", bufs=3, space="PSUM") as opool,
            tc.tile_pool(name="ppool", bufs=1, space="PSUM") as ppool,
            tc.tile_pool(name="dpool", bufs=1, space="DRAM") as dpool,
        ):
            # ---- constants (wqk first; the rest load after x batch 0) ----
            wqk_sb = cpool.tile([128, FS, NDT, 128], BF16)
            nc.sync.dma_start(wqk_sb[:], wqk.ap().rearrange("f dt p m -> p f dt m"))
            _deferred_consts = []
            _orig_dma = nc.sync.dma_start

            def _defer_dma(*a, **k):
                _deferred_consts.append((a, k))
            nc_sync_dma = _defer_dma
            wv_sb = cpool.tile([128, NDT, DV], BF16)
            _defer_dma(wv_sb[:], wv.ap().rearrange("dt p m -> p dt m"))
            w1_sb = cpool.tile([DV, 64], BF16)
            _defer_dma(w1_sb[:], w1.ap())
            w2_sb = cpool.tile([64, 64], BF16)
            _defer_dma(w2_sb[:], w2.ap())
            bqk_sb = cpool.tile([128, 1], F32)
            _defer_dma(bqk_sb[:], bqk.ap())
            bv_sb = cpool.tile([64, 1], F32)
            _defer_dma(bv_sb[:], bv.ap())
            b1_sb = cpool.tile([64, 1], F32)
            _defer_dma(b1_sb[:], b1.ap())
            b2_sb = cpool.tile([64, 1], F32)
            _defer_dma(b2_sb[:], b2.ap())
            maskf_sb = cpool.tile([128, 128], F32)
            make_upper_triangular(nc, maskf_sb[:], val=1.0, diag=True)
            mask_sb = cpool.tile([128, 128], BF16)
            nc.vector.tensor_copy(mask_sb[:], maskf_sb[:])
            identf_sb = cpool.tile([64, 64], F32)
            make_identity(nc, identf_sb[:])

            NRS = 2 * B  # one reduce-scatter per half-batch (1024 tokens)
            QT = 1024 // NCORES  # tokens per core per reduce-scatter (128)
            rs_in = [dpool.tile([NCORES, 64, QT], F32, name=f"rs_in{i}",
                                uniquify=False) for i in range(NRS)]
            rs_out = [dpool.tile([64, QT], F32, name=f"rs_out{i}",
                                 uniquify=False) for i in range(NRS)]
            rs_left = {i: 2 for i in range(NRS)}

            def epilogue(q):
                """relu(z + b1) @ W2 + b2 on reduce-scattered quarter q."""
                zin = spool.tile([64, QT], F32, tag="zin")
                nc.sync.dma_start(zin[:], rs_out[q][:])
                z = spool.tile([64, QT], BF16, tag="z")
                nc.vector.tensor_scalar(
                    z[:], zin[:], b1_sb[:], 0.0,
                    op0=mybir.AluOpType.add, op1=mybir.AluOpType.max)
                f_ps = stpool.tile([64, QT], F32, tag="st")
                nc.tensor.matmul(f_ps[:], w2_sb[:], z[:], start=True, stop=True)
                fin = spool.tile([64, QT], F32, tag="fin")
                nc.vector.tensor_scalar_add(fin[:], f_ps[:], b2_sb[:])
                out_sb = spool.tile([128, QT // 128, DV], F32, tag="outsb")
                for tt in range(QT // 128):
                    tr_ps = stpool.tile([128, 64], F32, tag="st")
                    nc.tensor.transpose(
                        tr_ps[:], fin[:, bass.ts(tt, 128)], identf_sb[:])
                    nc.vector.tensor_copy(out_sb[:, tt, :], tr_ps[:])

                nc.sync.dma_start(
                    out.ap()[bass.ds(q * QT, QT)].rearrange(
                        "(tt p) e -> p tt e", p=128),
                    out_sb[:])

            # The PE queue is strictly in-order, so a matmul that depends on
            # the (ACT/DMA/DVE) softmax-normalization chain must be emitted a
            # couple of pair-iterations later than the chain or it stalls the
            # whole queue.  pending holds [age, closure] entries.
            pending = []
            epi_queue = []  # (hb, closure) — flushed 2 batches later

            def flush_epilogues(upto_hb):
                keep = []
                for hb, fn in epi_queue:
                    if hb < upto_hb:
                        fn()
                    else:
                        keep.append((hb, fn))
                epi_queue[:] = keep

            def flush_pending(extra_age):
                keep = []
                for item in pending:
                    if item[0] + extra_age >= item[2]:
                        item[1]()
                    else:
                        keep.append(item)
                pending[:] = keep

            def age_pending():
                for item in pending:
                    item[0] += 1

            xtp_tiles = {}

            def load_xtp(b):
                t_ = xpool.tile([128, NDT, TP], BF16, name=f"xtp_sb{b}", tag="xtp")
                src = xtp.ap()[b].rearrange("(dt p) t -> p dt t", p=128)
                for dt_ in range(NDT):
                    nc.sync.dma_start(t_[:, dt_], src[:, dt_])
                xtp_tiles[b] = t_

            warm_ps = stpool.tile([128, 512], F32, tag="st")
            for wi in range(10):
                nc.tensor.matmul(
                    warm_ps[:], wqk_sb[:, 0, 0, :], wqk_sb[:, 0, :, 0:128],
                    start=(wi == 0), stop=(wi == 9))
            warm_sb = cpool.tile([1, 1], F32)
            nc.vector.tensor_copy(warm_sb[:], warm_ps[0:1, 0:1])
            nc.sync.dma_start(out.ap()[0:1, 0:1].bitcast(F32), warm_sb[:])

            load_xtp(0)
            for a_, k_ in _deferred_consts:
                _orig_dma(*a_, **k_)
            _deferred_consts.clear()
            for b in range(B):
                xtp_sb = xtp_tiles.pop(b)

                # ---- QK projection: rows 0:64 = Q^T, rows 64:128 = K^T ----
                qkt_sb = qkpool.tile([128, T], BF16, tag="qkt")
                qk2_sb = qkpool.tile([128, T], BF16, tag="qk2")  # swapped halves
                # weights outermost so consecutive matmuls share the
                # stationary operand (the redundant LDWEIGHTS is elided);
                # 4 token-chunk accumulators live in 2 wide psum tiles
                qk_ps = [stpool.tile([128, 1024], F32, tag="st",
                                     name=f"qk_ps{b}_{i}") for i in range(2)]
                n_w = FS * NDT
                i = 0
                for f in range(FS):
                    for dt_ in range(NDT):
                        for tci in range(4):
                            nc.tensor.matmul(
                                qk_ps[tci // 2][:, bass.ts(tci % 2, 512)],
                                wqk_sb[:, f, dt_, :],
                                xtp_sb[:, dt_, tci * 512 + f: tci * 512 + f + 512],
                                start=(i == 0), stop=(i == n_w - 1))
                        i += 1
                        if i == 2:
                            flush_pending(1)
                        elif i == 4:
                            flush_pending(99)
                            flush_epilogues(2 * (b - 1))
                for half in range(2):
                    sl = bass.ts(half, 1024)
                    nc.vector.tensor_scalar_add(
                        qkt_sb[:, sl], qk_ps[half][:], bqk_sb[:])
                    nc.sync.dma_start(qk2_sb[64:128, sl], qkt_sb[0:64, sl])
                    nc.sync.dma_start(qk2_sb[0:64, sl], qkt_sb[64:128, sl])

                # ---- V^T then transpose to [t, 65] with trailing ones ----
                vt_sb = vtpool.tile([64, T], F32)
                vt_ps = [stpool.tile([64, 1024], F32, tag="st",
                                     name=f"vt_ps{b}_{i}") for i in range(2)]
                for dt_ in range(NDT):
                    for tci in range(4):
                        nc.tensor.matmul(
                            vt_ps[tci // 2][:, bass.ts(tci % 2, 512)],
                            wv_sb[:, dt_, :],
                            xtp_sb[:, dt_, tci * 512 + 2: tci * 512 + 2 + 512],
                            start=(dt_ == 0), stop=(dt_ == NDT - 1))
                for half in range(2):
                    nc.vector.tensor_scalar_add(
                        vt_sb[:, bass.ts(half, 1024)], vt_ps[half][:], bv_sb[:])
                if b + 1 < B:
                    load_xtp(b + 1)
                v_sb = vpool.tile([128, NTT, DV + 1], BF16)
                nc.gpsimd.memset(v_sb[:, :, DV:DV + 1].bitcast(mybir.dt.uint16),
                                 0x3F80)  # bf16 1.0
                for tt in range(NTT):
                    tr_ps = stpool.tile([128, 64], F32, tag="st")
                    nc.tensor.transpose(
                        tr_ps[:], vt_sb[:, bass.ts(tt, 128)], identf_sb[:])
                    nc.vector.tensor_copy(v_sb[:, tt, 0:DV], tr_ps[:])

                # ---- attention, two q-passes of 1024 columns ----
                attnT_sb = atpool.tile([64, T], BF16, tag="attnT")
                p1_sb = atpool.tile([64, T], F32, tag="p1")
                for ps in range(2):
                    qlo, qhi = ps * 1024, ps * 1024 + 1024
                    o_ps = [opool.tile([DV + 1, 512], F32, tag="o",
                                       name=f"o_ps_{b}_{ps}_{c}")
                            for c in range(2)]
                    nkb = qhi // 128

                    def do_pv(strips, o_ps=o_ps, qlo=qlo, b=b):
                        """PV matmuls for a pair of exp'd strips; returns the
                        chunks whose accumulation closed."""
                        closed = []
                        for kb, qs, w, pt in strips:
                            for qc in range(2):
                                clo = qlo + qc * 512
                                chi = clo + 512
                                lo = max(clo, qs)
                                n = chi - lo
                                if n <= 0:
                                    continue
                                stop = kb == chi // 128 - 1
                                nc.tensor.matmul(
                                    o_ps[qc][:, lo - clo: lo - clo + n],
                                    v_sb[:, kb, :],
                                    pt[:, lo - qs: lo - qs + n],
                                    start=(kb == 0),
                                    stop=stop)
                                if stop:
                                    closed.append(qc)
                        return closed

                    prev_strips = None
                    for kba in list(range(0, nkb, 2)) + [None]:
                        age_pending()
                        flush_pending(0)
                        strips = []
                        if kba is not None:
                            meta = []
                            for kb in (kba, kba + 1):
                                qs = max(qlo, kb * 128)
                                w = qhi - qs
                                pt = ptpool.tile([128, 1024], BF16, tag="pt")
                                st_ps = stpool.tile([128, 1024], F32, tag="st")
                                meta.append((kb, qs, w, pt, st_ps))
                            # interleave the two strips' QK matmuls so the
                            # row-packed halves (rows 0:64 vs 64:128) overlap
                            wmax = max(m[2] for m in meta)
                            for c0 in range(0, wmax, 512):
                                for kb, qs, w, pt, st_ps in meta:
                                    if c0 >= w:
                                        continue
                                    cw = min(512, w - c0)
                                    if kb % 2 == 0:
                                        lhsT = qk2_sb[0:64, bass.ts(kb, 128)]
                                        rhs = qkt_sb[0:64, qs + c0: qs + c0 + cw]
                                    else:
                                        lhsT = qkt_sb[64:128, bass.ts(kb, 128)]
                                        rhs = qk2_sb[64:128, qs + c0: qs + c0 + cw]
                                    nc.tensor.matmul(
                                        st_ps[:, c0:c0 + cw], lhsT, rhs,
                                        start=True, stop=True)
                            for kb, qs, w, pt, st_ps in meta:
                                nc.scalar.activation(
                                    pt[:, 0:w], st_ps[:, 0:w], EXP, scale=0.125)
                                if kb * 128 >= qlo:
                                    nc.vector.tensor_mul(
                                        pt[:, 0:128], pt[:, 0:128], mask_sb[:])
                                strips.append((kb, qs, w, pt))
                        # PV lags one pair-iteration so it never waits on exp
                        closed = do_pv(prev_strips) if prev_strips else []
                        prev_strips = strips
                        # early per-chunk normalization + W1 partial as soon
                        # as a chunk's accumulation group closes, so ACT/DVE/
                        # DMA work overlaps the remaining PV matmuls.
                        for qc in closed:
                            clo = qlo + qc * 512
                            # l -> [128, 4] across partitions for a fast
                            # reciprocal, then back and broadcast
                            l_sb = spool.tile([128, 512], F32, tag="l",
                                              padded_shape=None)
                            nc.scalar.copy(l_sb[DV:DV + 1, :],
                                           o_ps[qc][DV:DV + 1, :])
                            lt_sb = spool.tile([128, 4], F32, tag="lt")
                            nc.sync.dma_start(lt_sb[:], l_sb[DV:DV + 1, :])
                            lti_sb = spool.tile([128, 4], F32, tag="lti")
                            nc.vector.reciprocal(lti_sb[:], lt_sb[:])
                            linv_sb = spool.tile([1, 512], F32, tag="linv")
                            nc.sync.dma_start(linv_sb[:], lti_sb[:])
                            lbc_sb = spool.tile([64, 512], F32, tag="lbc")
                            nc.gpsimd.partition_broadcast(lbc_sb[:], linv_sb[:])
                            nc.vector.tensor_mul(
                                attnT_sb[:, clo:clo + 512],
                                o_ps[qc][0:DV, :], lbc_sb[:])

                            def p1_work(clo=clo, b=b, attnT_sb=attnT_sb,
                                        p1_sb=p1_sb):
                                p1_ps = ppool.tile([64, 512], F32, tag="p1ps")
                                nc.tensor.matmul(
                                    p1_ps[:], w1_sb[:],
                                    attnT_sb[:, clo:clo + 512],
                                    start=True, stop=True)
                                nc.vector.tensor_copy(
                                    p1_sb[:, clo:clo + 512], p1_ps[:])
                                hb = 2 * b + clo // 1024
                                for hh in range(512 // QT):
                                    s0 = clo + QT * hh
                                    nc.sync.dma_start(
                                        rs_in[hb][(s0 % 1024) // QT],
                                        p1_sb[:, s0:s0 + QT])
                                rs_left[hb] -= 1
                                if rs_left[hb] == 0:
                                    nc.gpsimd.collective_compute(
                                        "ReduceScatter",
                                        mybir.AluOpType.add,
                                        replica_groups=[list(range(NCORES))],
                                        ins=[rs_in[hb].opt()],
                                        outs=[rs_out[hb].opt()],
                                    )
                                    epi_queue.append(
                                        (hb, lambda hb=hb: epilogue(hb)))
                            pending.append([0, p1_work, 2])

            flush_pending(99)
            flush_epilogues(NRS)

    nc.compile()
    return nc


def _get_nc():
    if "nc" not in _STATE:
        _STATE["nc"] = _build()
    return _STATE["nc"]


def _prep_inputs(x, Wq, bq, Wk, bk, Wv, bv, W1, b1, W2, b2):
    f = np.float32
    x = np.ascontiguousarray(np.asarray(x, f))
    xtp = np.zeros((B, D, TP), f)
    xtp[:, :, FS - 1:] = x.transpose(0, 2, 1)
    Wq = np.asarray(Wq, f)
    Wk = np.asarray(Wk, f)
    Wv = np.asarray(Wv, f)
    W1 = np.asarray(W1, f)
    W2 = np.asarray(W2, f)
    bq = np.asarray(bq, f)
    bk = np.asarray(bk, f)
    bv = np.asarray(bv, f)
    b1 = np.asarray(b1, f)
    b2 = np.asarray(b2, f)

    bf = ml_dtypes.bfloat16
    xtp = np.ascontiguousarray(xtp.astype(bf))
    in_maps = []
    for c in range(NCORES):
        hs = slice(c * DK, (c + 1) * DK)
        wqk_c = np.concatenate([Wq[:, :, hs], Wk[:, :, hs]], axis=2)
        in_maps.append({
            "xtp": xtp,
            "wqk": np.ascontiguousarray(
                wqk_c.reshape(FS, NDT, 128, 128).astype(bf)),
            "wv": np.ascontiguousarray(Wv[:, hs].reshape(NDT, 128, DV).astype(bf)),
            "w1": np.ascontiguousarray(W1[hs, :].astype(bf)),
            "w2": np.ascontiguousarray(W2.astype(bf)),
            "bqk": np.ascontiguousarray(
                np.concatenate([bq[hs], bk[hs]])[:, None]),
            "bv": np.ascontiguousarray(bv[hs][:, None]),
            "b1": np.ascontiguousarray(b1[:, None]),
            "b2": np.ascontiguousarray(b2[:, None]),
        })
    return in_maps


def _run(inputs, trace=False):
    nc = _get_nc()
    in_maps = _prep_inputs(**inputs)
    last_exc = None
    for attempt in range(3):
        try:
            r = bass_utils.run_bass_kernel_spmd(
                nc, in_maps, core_ids=list(range(NCORES)), trace=trace)
            break
        except Exception as exc:  # rare transient NRT exec-unit failures
            last_exc = exc
            import jax
            try:
                jax.clear_caches()
            except Exception:
                pass
            try:
                from concourse import bass2jax
                bass2jax._bass_exec_cache.clear()
            except Exception:
                pass
            _STATE.clear()
            nc = _get_nc()
    else:
        raise last_exc
    # Reduce-scatter e covers half-batch e (1024 tokens); core c owns
    # tokens [128c, 128c+128) of it, stored as eighth e of its output.
    QT = 1024 // NCORES
    full = np.empty((B * T, DV), np.float32)
    for c in range(NCORES):
        oc = r.results[c]["out"]
        for e in range(2 * B):
            full[1024 * e + QT * c: 1024 * e + QT * (c + 1)] = \
                oc[QT * e: QT * (e + 1)]
    full = full.reshape(B, T, DV)
    return full, r


def kernel(**inputs):
    full, _ = _run(inputs, trace=False)
    return full

